# revision 1
# baseline (speedup 1.0000x reference)
"""GuidedAttentionLoss on Trainium2 — 8 NeuronCores, batch-parallel.

loss = mean(attention_weights * mask), mask[b,i,j] =
    (i < out_len_b) & (j < in_len_b) ? exp(-(j - floor(i/out*in))^2 / (2*0.4^2)) : 0

Strategy: shard B=64 across 8 cores (8 batches each). Each core streams its
25.6MB attention shard through SBUF in [128, 400] row-tiles, generates the
mask on the fly and does a fused multiply+reduce. Per-(b,i) scalars
(ideal_j etc.) are precomputed on host (tiny [B,T] work) and fed as f32
tables; validity is folded into the exp argument so no per-element select
ops are needed:

  g = exp(-3.125 * (j - ideal)^2)  with  -3.125 == -1/(2*sigma^2) exactly
  row-invalid  -> ideal := -1e4           => exp arg <= -3.1e8 => g = 0
  col-invalid  -> j     := j + 1e4 (A) / arg += -3.125e7 (B)   => g = 0

Two per-tile mask pipelines, interleaved to balance engines:
  A: ACT Square(jc - ideal) ; ACT Exp(-3.125*d2)
  B: DVE/GPSIMD stt t = j*(-2*ideal) + (j^2 + colpen) ; ACT Exp(-3.125*t - 3.125*ideal^2)
Both end with DVE tensor_tensor_reduce (g*attn, sum) -> acc[:, tile].
Host sums the 8 [128,128] per-core accumulators for the global mean.
"""

import numpy as np

import concourse.bacc as bacc
import concourse.bass as bass  # noqa: F401  (AP types etc.)
import concourse.mybir as mybir
from concourse import tile
from concourse.bass_utils import run_bass_kernel_spmd

N_CORES = 8
B, T, E = 64, 2000, 400
B_LOC = B // N_CORES  # 8 batches per core
P = 128
NT = 16  # row-tiles per batch: 15 full + 1 tail overlapping (rows 1872..1999)
NTILES = B_LOC * NT  # 128 tiles per core
NEG_SCALE = -3.125  # -1/(2*sigma^2), exact in binary fp
F32 = mybir.dt.float32
AF = mybir.ActivationFunctionType
OP = mybir.AluOpType

_TILE_I0 = [min(ti * P, T - P) for ti in range(NT)]

# per-tile mask-gen engine: A = ACT Square+Exp, B = DVE stt + ACT Exp,
# C = GPSIMD stt + ACT Exp.  Tune mix from trace.
TYPE_PATTERN = (["A", "B"] * NT)[:NT]

_NC_CACHE = {}


def _build_nc(rep=1):
    nc = bacc.Bacc(None, target_bir_lowering=False)
    attn = nc.declare_dram_parameter("attn", [B_LOC, T, E], F32, isOutput=False)
    negideal_d = nc.declare_dram_parameter("negideal", [P, NTILES], F32, isOutput=False)
    m2i_d = nc.declare_dram_parameter("m2i", [P, NTILES], F32, isOutput=False)
    n3i2_d = nc.declare_dram_parameter("n3i2", [P, NTILES], F32, isOutput=False)
    inlen_d = nc.declare_dram_parameter("inlen", [P, B_LOC], F32, isOutput=False)
    acc_d = nc.declare_dram_parameter("acc", [P, NTILES], F32, isOutput=True)

    with tile.TileContext(nc) as tc:
        with (
            tc.tile_pool(name="const", bufs=1) as const_pool,
            tc.tile_pool(name="batch", bufs=2) as batch_pool,
            tc.tile_pool(name="attn", bufs=4) as attn_pool,
            tc.tile_pool(name="work", bufs=4) as work_pool,
            tc.tile_pool(name="g", bufs=4) as g_pool,
            tc.tile_pool(name="junk", bufs=4) as junk_pool,
        ):
            j_i32 = const_pool.tile([P, E], mybir.dt.int32, tag="j_i32")
            j_f32 = const_pool.tile([P, E], F32, tag="j_f32")
            j2 = const_pool.tile([P, E], F32, tag="j2")
            negideal = const_pool.tile([P, NTILES], F32, tag="negideal")
            m2i = const_pool.tile([P, NTILES], F32, tag="m2i")
            n3i2 = const_pool.tile([P, NTILES], F32, tag="n3i2")
            inlen = const_pool.tile([P, B_LOC], F32, tag="inlen")
            acc = const_pool.tile([P, NTILES], F32, tag="acc")

            nc.gpsimd.iota(j_i32[:], pattern=[[1, E]], base=0, channel_multiplier=0)
            nc.vector.tensor_copy(j_f32[:], j_i32[:])
            nc.vector.tensor_tensor(j2[:], j_f32[:], j_f32[:], OP.mult)
            nc.sync.dma_start(out=negideal[:], in_=negideal_d[:])
            nc.sync.dma_start(out=m2i[:], in_=m2i_d[:])
            nc.sync.dma_start(out=n3i2[:], in_=n3i2_d[:])
            nc.sync.dma_start(out=inlen[:], in_=inlen_d[:])

            for _r, lb in ((r, b) for r in range(rep) for b in range(B_LOC)):
                # col-invalid (j >= in_len) penalties, built once per batch
                cm = batch_pool.tile([P, E], F32, tag="cm")
                nc.vector.tensor_scalar(
                    cm[:], j_f32[:], inlen[:, lb : lb + 1], None, OP.is_ge
                )
                jc = batch_pool.tile([P, E], F32, tag="jc")  # j + 1e4*colinv
                nc.vector.scalar_tensor_tensor(
                    jc[:], cm[:], 1e4, j_f32[:], OP.mult, OP.add
                )
                vcol = batch_pool.tile([P, E], F32, tag="vcol")  # j^2 + 1e7*colinv
                nc.vector.scalar_tensor_tensor(
                    vcol[:], cm[:], 1e7, j2[:], OP.mult, OP.add
                )
                for ti in range(NT):
                    col = lb * NT + ti
                    i0 = _TILE_I0[ti]
                    at = attn_pool.tile([P, E], F32, tag="at")
                    nc.sync.dma_start(out=at[:], in_=attn[lb, i0 : i0 + P, :])
                    g = g_pool.tile([P, E], F32, tag="g")
                    ty = TYPE_PATTERN[ti]
                    if ty == "A":
                        d2 = work_pool.tile([P, E], F32, tag="d2")
                        nc.scalar.activation(
                            d2[:],
                            jc[:],
                            AF.Square,
                            bias=negideal[:, col : col + 1],
                            scale=1.0,
                        )
                        nc.scalar.activation(g[:], d2[:], AF.Exp, scale=NEG_SCALE)
                    else:
                        tt = work_pool.tile([P, E], F32, tag="d2")
                        eng = nc.vector if ty == "B" else nc.gpsimd
                        eng.scalar_tensor_tensor(
                            tt[:],
                            j_f32[:],
                            m2i[:, col : col + 1],
                            vcol[:],
                            OP.mult,
                            OP.add,
                        )
                        nc.scalar.activation(
                            g[:],
                            tt[:],
                            AF.Exp,
                            bias=n3i2[:, col : col + 1],
                            scale=NEG_SCALE,
                        )
                    jk = junk_pool.tile([P, E], F32, tag="jk")
                    # fused (g*attn) + row-sum; tensor_tensor_reduce dies at
                    # runtime in this environment, stt+accum_out is equivalent
                    nc.vector.scalar_tensor_tensor(
                        jk[:],
                        g[:],
                        1.0,
                        at[:],
                        OP.mult,
                        OP.mult,
                        accum_out=acc[:, col : col + 1],
                    )
            nc.sync.dma_start(out=acc_d[:], in_=acc[:])
    return nc


def _get_nc(rep=1):
    if rep not in _NC_CACHE:
        nc = _build_nc(rep)
        if not nc.is_finalized():
            nc.finalize()  # runs Bacc passes (wait splitting, reg alloc, ...)
        _NC_CACHE[rep] = nc
    return _NC_CACHE[rep]


def _make_tables(input_lengths, output_lengths, core):
    sl = slice(core * B_LOC, (core + 1) * B_LOC)
    in_len = np.asarray(input_lengths[sl], dtype=np.float32)
    out_len_i = np.asarray(output_lengths[sl], dtype=np.int64)
    safe_out = np.maximum(np.asarray(output_lengths[sl], dtype=np.float32), np.float32(1.0))
    negideal = np.empty((P, NTILES), np.float32)
    m2i = np.empty((P, NTILES), np.float32)
    n3i2 = np.empty((P, NTILES), np.float32)
    inlen = np.repeat(in_len[None, :], P, axis=0).astype(np.float32)
    p = np.arange(P, dtype=np.int64)
    for lb in range(B_LOC):
        for ti in range(NT):
            i0 = _TILE_I0[ti]
            i = i0 + p
            # replicate the reference's f32 arithmetic exactly
            i_f = i.astype(np.float32)
            ideal = np.floor((i_f / safe_out[lb]) * in_len[lb]).astype(np.float32)
            valid = (i < out_len_i[lb]) & (i >= ti * P)  # tail tile overlap dedup
            ideal_eff = np.where(valid, ideal, np.float32(-1e4)).astype(np.float32)
            col = lb * NT + ti
            negideal[:, col] = -ideal_eff
            m2i[:, col] = np.float32(-2.0) * ideal_eff
            n3i2[:, col] = (
                np.float64(-3.125) * ideal_eff.astype(np.float64) ** 2
            ).astype(np.float32)
    return {"negideal": negideal, "m2i": m2i, "n3i2": n3i2, "inlen": inlen}


def _run(attention_weights, input_lengths, output_lengths, **spmd_kwargs):
    attention_weights = np.ascontiguousarray(attention_weights, dtype=np.float32)
    in_maps = []
    for c in range(N_CORES):
        in_maps.append(
            {
                "attn": np.ascontiguousarray(
                    attention_weights[c * B_LOC : (c + 1) * B_LOC]
                ),
                **_make_tables(input_lengths, output_lengths, c),
            }
        )
    res = run_bass_kernel_spmd(_get_nc(), in_maps, list(range(N_CORES)), **spmd_kwargs)
    total = sum(float(r["acc"].sum(dtype=np.float64)) for r in res.results)
    return np.float32(total / float(B * T * E)), res


def kernel(attention_weights, input_lengths, output_lengths):
    out, _ = _run(attention_weights, input_lengths, output_lengths)
    return out



# revision 7
# speedup vs baseline: 1.2667x; 1.2667x over previous
"""GuidedAttentionLoss on Trainium2 — 8 NeuronCores, batch-parallel, row-skip.

loss = mean(attention_weights * mask), mask[b,i,j] =
    (i < out_len_b) & (j < in_len_b) ? exp(-(j - floor(i/out*in))^2 / (2*0.4^2)) : 0

Rows i >= out_len contribute nothing, so they are never loaded: batches are
sorted by out_len and dealt into 8 slot-groups of 8 (one batch per core per
slot); every core runs the identical tile schedule sized by the group max.
Attention is viewed as [8, 500, 1600] (4 rows per partition line) so each
DMA descriptor moves 6400 contiguous bytes; one DMA instruction per
supertile of up to 512 rows.

Per 400-wide sub-slice (one row per partition) the mask is generated as
  A: ACT Square(jc + (-ideal)) ; ACT Exp(-3.125 * d2)
  B: GPSIMD stt t = j*(-2*ideal) + (j^2 + colpen) ; ACT Exp(-3.125*t - 3.125*ideal^2)
then DVE stt (g*attn, accum) -> acc column. Host sums acc in f64.
Row-invalid rows get ideal=-1e4 (g underflows to 0); col-invalid (j>=in_len)
is folded into jc / vcol per slot.
"""

import numpy as np

import concourse.bacc as bacc
import concourse.bass as bass  # noqa: F401
import concourse.mybir as mybir
from concourse import tile
from concourse.bass_utils import run_bass_kernel_spmd

N_CORES = 8
B, T, E = 64, 2000, 400
B_LOC = B // N_CORES
P = 128
R4 = T // 4  # 500 row-groups of 4 rows
NEG_SCALE = -3.125  # -1/(2*sigma^2), exact in binary fp
F32 = mybir.dt.float32
AF = mybir.ActivationFunctionType
OP = mybir.AluOpType

# fraction of sub-slices using form A (ACT square) vs B (DVE stt);
# pattern indexed by running sub-slice counter; x_A = 7/8 balances
# ACT = 0.833*(1+x) against DVE = 1.042*(2-x) + slot-prep time
FORM_PATTERN = ["A", "A", "A", "A", "A", "A", "A", "B"]

_NC_CACHE = {}


def _schedule(out_lengths):
    """Sort batches by out_len desc, deal into 8 slots of 8 (slot g gets
    ranks [8g, 8g+8)). Returns (assign, tiles):
      assign[c][g] = global batch index held by core c at slot g
      tiles = list of (g, st, r) supertiles: slot g, row-group start st*128,
              r partitions (each = 4 rows)
    """
    order = np.argsort(-np.asarray(out_lengths), kind="stable")
    assign = [[int(order[8 * g + c]) for g in range(8)] for c in range(8)]
    tiles = []
    for g in range(8):
        max_out = int(np.asarray(out_lengths)[order[8 * g]])
        ngroups = (min(max_out, T) + 3) // 4  # row-groups of 4 needed
        st = 0
        while st * P < ngroups:
            r = min(P, ngroups - st * P)
            tiles.append((g, st, r))
            st += 1
    return assign, tiles


def _build_nc(tiles):
    ncols = 4 * len(tiles)
    nc = bacc.Bacc(None, target_bir_lowering=False)
    attn = nc.declare_dram_parameter("attn", [B_LOC, R4, 1600], F32, isOutput=False)
    negideal_d = nc.declare_dram_parameter("negideal", [P, ncols], F32, isOutput=False)
    m2i_d = nc.declare_dram_parameter("m2i", [P, ncols], F32, isOutput=False)
    n3i2_d = nc.declare_dram_parameter("n3i2", [P, ncols], F32, isOutput=False)
    inlen_d = nc.declare_dram_parameter("inlen", [P, B_LOC], F32, isOutput=False)
    acc_d = nc.declare_dram_parameter("acc", [P, ncols], F32, isOutput=True)

    with tile.TileContext(nc) as tc:
        with (
            tc.tile_pool(name="const", bufs=1) as const_pool,
            tc.tile_pool(name="slot", bufs=2) as slot_pool,
            tc.tile_pool(name="attn", bufs=3) as attn_pool,
            tc.tile_pool(name="work", bufs=6) as work_pool,
            tc.tile_pool(name="g", bufs=6) as g_pool,
            tc.tile_pool(name="junk", bufs=6) as junk_pool,
        ):
            j_i32 = const_pool.tile([P, E], mybir.dt.int32, tag="j_i32")
            j_f32 = const_pool.tile([P, E], F32, tag="j_f32")
            j2 = const_pool.tile([P, E], F32, tag="j2")
            negideal = const_pool.tile([P, ncols], F32, tag="negideal")
            m2i = const_pool.tile([P, ncols], F32, tag="m2i")
            n3i2 = const_pool.tile([P, ncols], F32, tag="n3i2")
            inlen = const_pool.tile([P, B_LOC], F32, tag="inlen")
            acc = const_pool.tile([P, ncols], F32, tag="acc")

            nc.gpsimd.iota(j_i32[:], pattern=[[1, E]], base=0, channel_multiplier=0)
            nc.vector.tensor_copy(j_f32[:], j_i32[:])
            nc.vector.tensor_tensor(j2[:], j_f32[:], j_f32[:], OP.mult)
            nc.gpsimd.memset(acc[:], 0.0)
            nc.sync.dma_start(out=negideal[:], in_=negideal_d[:])
            nc.sync.dma_start(out=m2i[:], in_=m2i_d[:])
            nc.sync.dma_start(out=n3i2[:], in_=n3i2_d[:])
            nc.sync.dma_start(out=inlen[:], in_=inlen_d[:])

            sub_i = 0
            cur_slot = -1
            jc = vcol = None
            for g, st, r in tiles:
                if g != cur_slot:
                    cur_slot = g
                    cm = slot_pool.tile([P, E], F32, tag="cm")
                    nc.vector.tensor_scalar(
                        cm[:], j_f32[:], inlen[:, g : g + 1], None, OP.is_ge
                    )
                    jc = slot_pool.tile([P, E], F32, tag="jc")
                    nc.vector.scalar_tensor_tensor(
                        jc[:], cm[:], 1e4, j_f32[:], OP.mult, OP.add
                    )
                    vcol = slot_pool.tile([P, E], F32, tag="vcol")
                    nc.vector.scalar_tensor_tensor(
                        vcol[:], cm[:], 1e7, j2[:], OP.mult, OP.add
                    )
                at = attn_pool.tile([P, 1600], F32, tag="at")
                nc.sync.dma_start(out=at[:r], in_=attn[g, st * P : st * P + r, :])
                for r4 in range(4):
                    c = sub_i * 4 + r4
                    gt = g_pool.tile([P, E], F32, tag="gt")
                    form = FORM_PATTERN[(sub_i + r4) % len(FORM_PATTERN)]
                    if form == "A":
                        d2 = work_pool.tile([P, E], F32, tag="d2")
                        nc.scalar.activation(
                            d2[:r],
                            jc[:r],
                            AF.Square,
                            bias=negideal[:r, c : c + 1],
                            scale=1.0,
                        )
                        nc.scalar.activation(gt[:r], d2[:r], AF.Exp, scale=NEG_SCALE)
                    else:
                        tt = work_pool.tile([P, E], F32, tag="d2")
                        nc.vector.scalar_tensor_tensor(
                            tt[:r],
                            j_f32[:r],
                            m2i[:r, c : c + 1],
                            vcol[:r],
                            OP.mult,
                            OP.add,
                        )
                        nc.scalar.activation(
                            gt[:r],
                            tt[:r],
                            AF.Exp,
                            bias=n3i2[:r, c : c + 1],
                            scale=NEG_SCALE,
                        )
                    jk = junk_pool.tile([P, E], F32, tag="jk")
                    nc.vector.scalar_tensor_tensor(
                        jk[:r],
                        gt[:r],
                        1.0,
                        at[:r, r4 * E : (r4 + 1) * E],
                        OP.mult,
                        OP.mult,
                        accum_out=acc[:r, c : c + 1],
                    )
                sub_i += 1
            nc.sync.dma_start(out=acc_d[:], in_=acc[:])
    return nc


def _get_nc(tiles):
    key = tuple(tiles)
    if key not in _NC_CACHE:
        nc = _build_nc(tiles)
        if not nc.is_finalized():
            nc.finalize()
        _NC_CACHE[key] = nc
    return _NC_CACHE[key]


def _make_tables(input_lengths, output_lengths, assign_c, tiles):
    """Per-core tables. assign_c[g] = batch index at slot g for this core."""
    ncols = 4 * len(tiles)
    negideal = np.full((P, ncols), 1e4, np.float32)  # -ideal_eff; invalid -> +1e4
    m2i = np.full((P, ncols), 2e4, np.float32)
    n3i2 = np.full((P, ncols), -3.125e8, np.float32)
    inlen = np.zeros((P, B_LOC), np.float32)
    il = np.asarray(input_lengths)
    ol = np.asarray(output_lengths)
    p = np.arange(P, dtype=np.int64)
    for g in range(8):
        b = assign_c[g]
        inlen[:, g] = np.float32(il[b])
        in_f = np.float32(il[b])
        out_i = int(ol[b])
        safe_out = np.float32(max(float(ol[b]), 1.0))
        for ti, (tg, st, r) in enumerate(tiles):
            if tg != g:
                continue
            for r4 in range(4):
                c = ti * 4 + r4
                i = (st * P + p) * 4 + r4  # global row per partition
                i_f = i.astype(np.float32)
                ideal = np.floor((i_f / safe_out) * in_f).astype(np.float32)
                valid = i < out_i
                ideal_eff = np.where(valid, ideal, np.float32(-1e4)).astype(np.float32)
                negideal[:, c] = -ideal_eff
                m2i[:, c] = np.float32(-2.0) * ideal_eff
                n3i2[:, c] = (
                    np.float64(-3.125) * ideal_eff.astype(np.float64) ** 2
                ).astype(np.float32)
    return {"negideal": negideal, "m2i": m2i, "n3i2": n3i2, "inlen": inlen}


def _run(attention_weights, input_lengths, output_lengths, **spmd_kwargs):
    attention_weights = np.ascontiguousarray(attention_weights, dtype=np.float32)
    assign, tiles = _schedule(output_lengths)
    in_maps = []
    for c in range(N_CORES):
        # core c's slot-g batch is assign[c][g]; gather its 8 batches in slot order
        shard = attention_weights[assign[c]].reshape(B_LOC, R4, 1600)
        in_maps.append(
            {
                "attn": np.ascontiguousarray(shard),
                **_make_tables(input_lengths, output_lengths, assign[c], tiles),
            }
        )
    res = run_bass_kernel_spmd(
        _get_nc(tiles), in_maps, list(range(N_CORES)), **spmd_kwargs
    )
    total = sum(float(r["acc"].sum(dtype=np.float64)) for r in res.results)
    return np.float32(total / float(B * T * E)), res


def kernel(attention_weights, input_lengths, output_lengths):
    out, _ = _run(attention_weights, input_lengths, output_lengths)
    return out


# revision 13
# speedup vs baseline: 2.1428x; 1.6917x over previous
"""GuidedAttentionLoss on Trainium2 — 8 NeuronCores, diagonal-band gather.

loss = mean(attention_weights * mask), mask[b,i,j] =
    (i < out_len_b) & (j < in_len_b) ? exp(-(j - floor(i/out*in))^2 / (2*0.4^2)) : 0

With sigma=0.4 the Gaussian underflows to exactly 0 in f32 beyond
|j - ideal_i| ~ 4.6, so per valid row only a ~9-wide band of columns can
contribute. Strategy:

- Batches are sorted by slope in/out and dealt into 8 slot-columns of 8
  (one batch per core per slot) -> pure SPMD: every core runs the identical
  program; per-core data (attention shard + mask tables) differs.
- Per column a quantized-affine "shear line" sigma(i) = a1*p1 + a2*p2 + at*t
  + b tracks ideal(i); a single 4-dim DMA access pattern
  [[3200+a1,16],[400+a2,8],[51200+at,nt],[1,W]] gathers the whole column's
  band ([128 rows/tile] x [W cols], nt tiles) in ONE DMA instruction.
  W is sized exactly on the host from the union of the 8 members' needs.
- Mask math per column, consolidated over the whole [128, nt*W] tile:
    d = w_iota - center   (DVE; center[p,t] = ideal - sigma, +1e4 if invalid)
    d2 = d*d              (ACT Square or DVE mult, alternating)
    g = exp(-3.125*d2)    (ACT)
    acc[:,s] += g*attn    (DVE stt accum)
  Garbage positions (front spill j<0, j>=min(in,400)) that land within 8 of a
  valid ideal are masked by per-tile threshold compare+fold ops; everything
  else dies in the Gaussian underflow. Host sums acc in f64.
"""

import numpy as np

import concourse.bacc as bacc
import concourse.bass as bass  # noqa: F401
import concourse.mybir as mybir
from concourse.ap import AP
from concourse import tile
from concourse.bass_utils import run_bass_kernel_spmd

N_CORES = 8
B, T, E = 64, 2000, 400
B_LOC = B // N_CORES
P = 128
D = 4       # band half-width kept exactly
PROX = 8    # garbage within this of a valid ideal must be masked
PADF = 512
PADB = 81920
FLAT = PADF + B_LOC * T * E + PADB
NEG_SCALE = -3.125
F32 = mybir.dt.float32
AF = mybir.ActivationFunctionType
OP = mybir.AluOpType

_NC_CACHE = {}


def _ideal_f32(i, in_len, out_len):
    safe_out = np.float32(max(float(out_len), 1.0))
    return np.floor((i.astype(np.float32) / safe_out) * np.float32(in_len)).astype(
        np.float32
    )


class _Seg:
    __slots__ = ("g", "members", "t0", "nt", "W", "mode", "a2", "at", "b",
                 "sigma", "flags", "sq_act")

    def key(self):
        return (self.g, self.t0, self.nt, self.W, self.mode, self.a2,
                self.at, self.b, tuple(self.flags), self.sq_act)


def _fit_segment(members, il, ol, t0, nt, g):
    """Fit shear line + W for rows [t0*128, (t0+nt)*128) of slot g."""
    seg = _Seg()
    seg.g = g
    seg.members = members
    seg.t0 = t0
    seg.nt = nt
    rows = nt * P
    i = t0 * P + np.arange(rows)
    A = np.full((8, rows), 1e9)
    Bb = np.full((8, rows), -1e9)
    valid = np.zeros((8, rows), bool)
    ideals = np.zeros((8, rows))
    for m, b in enumerate(members):
        o, n = int(ol[b]), int(il[b])
        valid[m] = i < min(o, T)
        idl = _ideal_f32(i, n, o).astype(np.float64)
        ideals[m] = idl
        A[m] = np.maximum(0.0, idl - D)
        Bb[m] = np.minimum(n - 1, idl + D)
    anyv = valid.any(0)
    Amin = np.where(valid, A, 1e9).min(0)
    Bmax = np.where(valid, Bb, -1e9).max(0)

    slopes = [il[b] / max(ol[b], 1) for b in members]
    cands = set()
    for s in set(np.quantile(slopes, [0.0, 0.25, 0.5, 0.75, 1.0])):
        for f1 in (np.floor, np.round):
            for f3 in (np.floor, np.round):
                at3 = int(f3(128 * s))
                for dat in (-1, 0, 1):
                    cands.add((int(f1(s)), at3 + dat))
    rr = np.arange(rows)
    t_idx = rr // P
    p = rr % P
    best = None
    for a2, at in cands:
        sig0 = a2 * p + at * t_idx
        b_off = int(np.floor((Amin - sig0)[anyv].min()))
        W = int(np.ceil((Bmax - sig0)[anyv].max() - b_off)) + 1
        if best is None or W < best[0]:
            best = (W, a2, at, b_off)
    WL = best[0]
    WS = int(Bmax[anyv].max()) + 1
    if WS <= WL:
        seg.mode = "S"
        seg.a2 = seg.at = 0
        seg.b = 0
        seg.W = WS
    else:
        seg.mode = "L"
        _, seg.a2, seg.at, seg.b = best
        seg.W = WL
    # sub-512B descriptors pay a 2x latency multiplier; W in (64,128) costs
    # the same as W=128, so round up for free coverage slack
    if 64 < seg.W < 128:
        seg.W = 128
    assert seg.W <= E + PROX, (seg.W, seg.mode)
    sig = seg.a2 * p + seg.at * t_idx + seg.b
    seg.sigma = sig

    flags = []
    if seg.mode == "S":
        needB = any(
            min(int(il[b]), E) < seg.W
            and (valid[m] & (ideals[m] >= min(int(il[b]), E) - PROX)).any()
            for m, b in enumerate(members)
        )
        if needB:
            flags.append((-1, "B"))  # consolidated over whole segment
    else:
        for t in range(nt):
            rs = slice(t * P, (t + 1) * P)
            needA = needB = False
            for m, b in enumerate(members):
                lim = min(int(il[b]), E)
                v = valid[m][rs]
                if not v.any():
                    continue
                idl = ideals[m][rs]
                sg = sig[rs]
                if ((sg < 0) & v & (idl <= PROX)).any():
                    needA = True
                if ((sg + seg.W > lim) & v & (idl >= lim - PROX)).any():
                    needB = True
            if needA:
                flags.append((t, "A"))
            if needB:
                flags.append((t, "B"))
    seg.flags = flags
    return seg


def _coverage_check(segs, il, ol):
    for seg in segs:
        rows = seg.nt * P
        i = seg.t0 * P + np.arange(rows)
        for m, b in enumerate(seg.members):
            o, n = int(ol[b]), int(il[b])
            v = i < min(o, T)
            if not v.any():
                continue
            idl = _ideal_f32(i, n, o).astype(np.float64)
            A = np.maximum(0.0, idl - D)
            Bb = np.minimum(n - 1, idl + D)
            ok = (~v) | ((seg.sigma <= A) & (Bb < seg.sigma + seg.W))
            assert ok.all(), (seg.g, b, np.where(~ok)[0][:5])
            # flat addressing bounds
            base = seg.g * T * E + i * E + seg.sigma
            assert (PADF + base).min() >= 0
            assert (PADF + base + seg.W).max() <= FLAT


def _build_schedule(input_lengths, output_lengths):
    il = np.asarray(input_lengths, dtype=np.int64)
    ol = np.asarray(output_lengths, dtype=np.int64)
    slopes = il.astype(np.float64) / np.maximum(ol, 1)
    order = np.argsort(slopes, kind="stable")
    assign = [[int(order[8 * g + c]) for g in range(8)] for c in range(8)]
    segs = []
    for g in range(8):
        members = [assign[c][g] for c in range(8)]
        max_out = max(int(ol[b]) for b in members)
        nt = (min(max_out, T) + P - 1) // P
        seg = _fit_segment(members, il, ol, 0, nt, g)
        segs.append(seg)
    for k, seg in enumerate(segs):
        seg.sq_act = k % 2 == 0
    _coverage_check(segs, il, ol)
    return assign, segs


def _build_nc(segs):
    ntt = sum(s.nt for s in segs)
    nf = sum(len(s.flags) for s in segs)
    nseg = len(segs)
    nc = bacc.Bacc(None, target_bir_lowering=False)
    attn = nc.declare_dram_parameter("attn", [FLAT], F32, isOutput=False)
    center_d = nc.declare_dram_parameter("center", [P, ntt], F32, isOutput=False)
    thr_d = nc.declare_dram_parameter("thr", [P, max(nf, 1)], F32, isOutput=False)
    acc_d = nc.declare_dram_parameter("acc", [P, nseg], F32, isOutput=True)

    with tile.TileContext(nc) as tc:
        with (
            tc.tile_pool(name="const", bufs=1) as const_pool,
            tc.tile_pool(name="at", bufs=3) as at_pool,
            tc.tile_pool(name="d", bufs=3) as d_pool,
            tc.tile_pool(name="g", bufs=3) as g_pool,
            tc.tile_pool(name="cm", bufs=3) as cm_pool,
            tc.tile_pool(name="junk", bufs=3) as junk_pool,
        ):
            w_i32 = const_pool.tile([P, E], mybir.dt.int32, tag="w_i32")
            w_f32 = const_pool.tile([P, E], F32, tag="w_f32")
            center = const_pool.tile([P, ntt], F32, tag="center")
            thr = const_pool.tile([P, max(nf, 1)], F32, tag="thr")
            acc = const_pool.tile([P, nseg], F32, tag="acc")

            nc.gpsimd.iota(w_i32[:], pattern=[[1, E]], base=0, channel_multiplier=0)
            nc.vector.tensor_copy(w_f32[:], w_i32[:])
            nc.gpsimd.memset(acc[:], 0.0)
            nc.sync.dma_start(out=center[:], in_=center_d[:])
            if nf:
                nc.sync.dma_start(out=thr[:], in_=thr_d[:])

            k0 = 0
            f0 = 0
            for si, seg in enumerate(segs):
                nt, W = seg.nt, seg.W
                fw = nt * W
                at = at_pool.tile([P, fw], F32, tag="at")
                src = AP(
                    attn[:].tensor,
                    PADF + seg.g * T * E + seg.t0 * P * E + seg.b,
                    [
                        [E + seg.a2, P],
                        [P * E + seg.at, nt],
                        [1, W],
                    ],
                )
                dst = at[:]
                nc.sync.dma_start(
                    out=AP(dst.tensor, dst.offset, [dst.ap[0], [W, nt], [1, W]]),
                    in_=src,
                )
                # d = w - center  (broadcast w over t, center over w)
                d = d_pool.tile([P, fw], F32, tag="d")
                wap = w_f32[:, 0:W]
                w_b = AP(wap.tensor, wap.offset, [wap.ap[0], [0, nt], [1, W]])
                cap = center[:, k0 : k0 + nt]
                c_b = AP(cap.tensor, cap.offset, [cap.ap[0], [1, nt], [0, W]])
                dap = d[:]
                d3 = AP(dap.tensor, dap.offset, [dap.ap[0], [W, nt], [1, W]])
                nc.vector.tensor_tensor(d3, w_b, c_b, OP.subtract)
                # threshold masks on flagged tiles
                for ti, kind in seg.flags:
                    cm_w = fw if ti < 0 else W
                    cm = cm_pool.tile([P, cm_w], F32, tag="cm")
                    if ti < 0:  # consolidated (mode S): compare w pattern
                        cmap = cm[:]
                        cm3 = AP(cmap.tensor, cmap.offset,
                                 [cmap.ap[0], [W, nt], [1, W]])
                        nc.vector.tensor_scalar(
                            cm3, w_b, thr[:, f0 : f0 + 1], None, OP.is_ge
                        )
                        dsl = d[:]
                    else:
                        op = OP.is_lt if kind == "A" else OP.is_ge
                        nc.vector.tensor_scalar(
                            cm[:], w_f32[:, 0:W], thr[:, f0 : f0 + 1], None, op
                        )
                        dsl = d[:, ti * W : (ti + 1) * W]
                    nc.vector.scalar_tensor_tensor(
                        dsl, cm[:, 0:cm_w] if ti < 0 else cm[:], 3e4, dsl,
                        OP.mult, OP.add,
                    )
                    f0 += 1
                # d2, g, reduce
                d2 = junk_pool.tile([P, fw], F32, tag="d2")
                if seg.sq_act:
                    nc.scalar.activation(d2[:], d[:], AF.Square)
                else:
                    nc.vector.tensor_tensor(d2[:], d[:], d[:], OP.mult)
                gt = g_pool.tile([P, fw], F32, tag="gt")
                nc.scalar.activation(gt[:], d2[:], AF.Exp, scale=NEG_SCALE)
                jk = junk_pool.tile([P, fw], F32, tag="jk")
                nc.vector.scalar_tensor_tensor(
                    jk[:], gt[:], 1.0, at[:], OP.mult, OP.mult,
                    accum_out=acc[:, si : si + 1],
                )
                k0 += nt
            nc.sync.dma_start(out=acc_d[:], in_=acc[:])
    return nc


def _get_nc(segs):
    key = tuple(s.key() for s in segs)
    if key not in _NC_CACHE:
        nc = _build_nc(segs)
        if not nc.is_finalized():
            nc.finalize()
        _NC_CACHE[key] = nc
    return _NC_CACHE[key]


def _make_tables(il, ol, assign_c, segs):
    ntt = sum(s.nt for s in segs)
    nf = sum(len(s.flags) for s in segs)
    center = np.full((P, ntt), 1e4, np.float32)
    thr = np.zeros((P, max(nf, 1)), np.float32)
    k0 = 0
    f0 = 0
    for seg in segs:
        b = assign_c[seg.g]
        o, n = int(ol[b]), int(il[b])
        lim = min(n, E)
        rows = seg.nt * P
        i = seg.t0 * P + np.arange(rows)
        idl = _ideal_f32(i, n, o)
        validr = i < min(o, T)
        cen = np.where(validr, idl - seg.sigma.astype(np.float32), np.float32(1e4))
        center[:, k0 : k0 + seg.nt] = cen.reshape(seg.nt, P).T
        for ti, kind in seg.flags:
            if ti < 0:  # consolidated mode-S B mask: sigma = 0
                thr[:, f0] = np.float32(lim)
            else:
                sg = seg.sigma[ti * P : (ti + 1) * P]
                if kind == "A":
                    thr[:, f0] = (-sg).astype(np.float32)
                else:
                    thr[:, f0] = (lim - sg).astype(np.float32)
            f0 += 1
        k0 += seg.nt
    return {"center": center, "thr": thr}


def _run(attention_weights, input_lengths, output_lengths, **spmd_kwargs):
    attention_weights = np.ascontiguousarray(attention_weights, dtype=np.float32)
    il = np.asarray(input_lengths, dtype=np.int64)
    ol = np.asarray(output_lengths, dtype=np.int64)
    assign, segs = _build_schedule(il, ol)
    in_maps = []
    for c in range(N_CORES):
        flat = np.empty(FLAT, np.float32)
        flat[:PADF] = 0.0
        flat[PADF : PADF + B_LOC * T * E] = attention_weights[assign[c]].reshape(-1)
        flat[PADF + B_LOC * T * E :] = 0.0
        in_maps.append(
            {"attn": flat, **_make_tables(il, ol, assign[c], segs)}
        )
    res = run_bass_kernel_spmd(
        _get_nc(segs), in_maps, list(range(N_CORES)), **spmd_kwargs
    )
    total = sum(float(r["acc"].sum(dtype=np.float64)) for r in res.results)
    return np.float32(total / float(B * T * E)), res


def kernel(attention_weights, input_lengths, output_lengths):
    out, _ = _run(attention_weights, input_lengths, output_lengths)
    return out


# revision 17
# speedup vs baseline: 2.7796x; 1.2972x over previous
"""GuidedAttentionLoss on Trainium2 — 8 NeuronCores, diagonal-band gather.

loss = mean(attention_weights * mask), mask[b,i,j] =
    (i < out_len_b) & (j < in_len_b) ? exp(-(j - floor(i/out*in))^2 / (2*0.4^2)) : 0

With sigma=0.4 the Gaussian underflows to exactly 0 in f32 beyond
|j - ideal_i| ~ 4.6, so per valid row only a ~9-wide band of columns can
contribute. Strategy:

- Batches are sorted by slope in/out and dealt into 8 slot-columns of 8
  (one batch per core per slot) -> pure SPMD: every core runs the identical
  program; per-core data (attention shard + mask tables) differs.
- Per column a quantized-affine "shear line" sigma(i) = a1*p1 + a2*p2 + at*t
  + b tracks ideal(i); a single 4-dim DMA access pattern
  [[3200+a1,16],[400+a2,8],[51200+at,nt],[1,W]] gathers the whole column's
  band ([128 rows/tile] x [W cols], nt tiles) in ONE DMA instruction.
  W is sized exactly on the host from the union of the 8 members' needs.
- Mask math per column, consolidated over the whole [128, nt*W] tile:
    d = w_iota - center   (DVE; center[p,t] = ideal - sigma, +1e4 if invalid)
    d2 = d*d              (ACT Square or DVE mult, alternating)
    g = exp(-3.125*d2)    (ACT)
    acc[:,s] += g*attn    (DVE stt accum)
  Garbage positions (front spill j<0, j>=min(in,400)) that land within 8 of a
  valid ideal are masked by per-tile threshold compare+fold ops; everything
  else dies in the Gaussian underflow. Host sums acc in f64.
"""

import numpy as np

import concourse.bacc as bacc
import concourse.bass as bass  # noqa: F401
import concourse.mybir as mybir
from concourse.ap import AP
from concourse import tile
from concourse.bass_utils import run_bass_kernel_spmd

N_CORES = 8
B, T, E = 64, 2000, 400
B_LOC = B // N_CORES
P = 128
D = 4       # band half-width kept exactly
PROX = 8    # garbage within this of a valid ideal must be masked
PADF = 512
PADB = 81920
FLAT = PADF + B_LOC * T * E + PADB
NEG_SCALE = -3.125
F32 = mybir.dt.float32
AF = mybir.ActivationFunctionType
OP = mybir.AluOpType

_NC_CACHE = {}


def _ideal_f32(i, in_len, out_len):
    safe_out = np.float32(max(float(out_len), 1.0))
    return np.floor((i.astype(np.float32) / safe_out) * np.float32(in_len)).astype(
        np.float32
    )


class _Seg:
    __slots__ = ("g", "members", "t0", "nt", "W", "mode", "a2", "at", "b",
                 "sigma", "flags", "sq_act")

    def key(self):
        return (self.g, self.t0, self.nt, self.W, self.mode, self.a2,
                self.at, self.b, tuple(self.flags), self.sq_act)


def _fit_segment(members, il, ol, t0, nt, g):
    """Fit shear line + W for rows [t0*128, (t0+nt)*128) of slot g."""
    seg = _Seg()
    seg.g = g
    seg.members = members
    seg.t0 = t0
    seg.nt = nt
    rows = nt * P
    i = t0 * P + np.arange(rows)
    A = np.full((8, rows), 1e9)
    Bb = np.full((8, rows), -1e9)
    valid = np.zeros((8, rows), bool)
    ideals = np.zeros((8, rows))
    for m, b in enumerate(members):
        o, n = int(ol[b]), int(il[b])
        valid[m] = i < min(o, T)
        idl = _ideal_f32(i, n, o).astype(np.float64)
        ideals[m] = idl
        A[m] = np.maximum(0.0, idl - D)
        Bb[m] = np.minimum(n - 1, idl + D)
    anyv = valid.any(0)
    Amin = np.where(valid, A, 1e9).min(0)
    Bmax = np.where(valid, Bb, -1e9).max(0)

    slopes = [il[b] / max(ol[b], 1) for b in members]
    cands = set()
    for s in set(np.quantile(slopes, [0.0, 0.25, 0.5, 0.75, 1.0])):
        for f1 in (np.floor, np.round):
            for f3 in (np.floor, np.round):
                at3 = int(f3(128 * s))
                for dat in (-1, 0, 1):
                    cands.add((int(f1(s)), at3 + dat))
    rr = np.arange(rows)
    t_idx = rr // P
    p = rr % P

    def _flag_tiles(sig, W):
        """Which tiles need A/B garbage masks under line sig, width W."""
        fa, fb = [], []
        for t in range(nt):
            rs = slice(t * P, (t + 1) * P)
            needA = needB = False
            for m, b in enumerate(members):
                lim = min(int(il[b]), E)
                v = valid[m][rs]
                if not v.any():
                    continue
                idl = ideals[m][rs]
                sg = sig[rs]
                if ((sg < 0) & v & (idl <= PROX)).any():
                    needA = True
                if ((sg + W > lim) & v & (idl >= lim - PROX)).any():
                    needB = True
            if needA:
                fa.append(t)
            if needB:
                fb.append(t)
        return fa, fb

    best = None
    for a2, at in cands:
        sig0 = a2 * p + at * t_idx
        b_off = int(np.floor((Amin - sig0)[anyv].min()))
        W = int(np.ceil((Bmax - sig0)[anyv].max() - b_off)) + 1
        fa, fb = _flag_tiles(sig0 + b_off, W)
        # cost: ~4ns/free-elem (compute+DMA) + ~1.2us per flagged tile
        cost = nt * W * 4.0 + (len(fa) + len(fb)) * 1200.0
        if best is None or cost < best[0]:
            best = (cost, W, a2, at, b_off, fa, fb)
    sigS = np.zeros(rows, dtype=np.int64)
    WS = int(Bmax[anyv].max()) + 1
    faS, fbS = _flag_tiles(sigS, WS)
    costS = nt * WS * 4.0 + (len(faS) + len(fbS)) * 1200.0
    if costS <= best[0]:
        seg.mode = "S"
        seg.a2 = seg.at = 0
        seg.b = 0
        seg.W = WS
        fa, fb = faS, fbS
    else:
        seg.mode = "L"
        _, seg.W, seg.a2, seg.at, seg.b, fa, fb = best
    assert seg.W <= E + PROX, (seg.W, seg.mode)
    seg.sigma = seg.a2 * p + seg.at * t_idx + seg.b

    def _runs(ts):
        out = []
        for t in ts:
            if out and t == out[-1][0] + out[-1][1]:
                out[-1] = (out[-1][0], out[-1][1] + 1)
            else:
                out.append((t, 1))
        return out

    seg.flags = [(t0r, ln, "A") for t0r, ln in _runs(fa)] + [
        (t0r, ln, "B") for t0r, ln in _runs(fb)
    ]
    return seg


def _coverage_check(segs, il, ol):
    for seg in segs:
        rows = seg.nt * P
        i = seg.t0 * P + np.arange(rows)
        for m, b in enumerate(seg.members):
            o, n = int(ol[b]), int(il[b])
            v = i < min(o, T)
            if not v.any():
                continue
            idl = _ideal_f32(i, n, o).astype(np.float64)
            A = np.maximum(0.0, idl - D)
            Bb = np.minimum(n - 1, idl + D)
            ok = (~v) | ((seg.sigma <= A) & (Bb < seg.sigma + seg.W))
            assert ok.all(), (seg.g, b, np.where(~ok)[0][:5])
            # flat addressing bounds
            base = seg.g * T * E + i * E + seg.sigma
            assert (PADF + base).min() >= 0
            assert (PADF + base + seg.W).max() <= FLAT


def _build_schedule(input_lengths, output_lengths):
    il = np.asarray(input_lengths, dtype=np.int64)
    ol = np.asarray(output_lengths, dtype=np.int64)
    slopes = il.astype(np.float64) / np.maximum(ol, 1)
    order = np.argsort(slopes, kind="stable")
    assign = [[int(order[8 * g + c]) for g in range(8)] for c in range(8)]
    segs = []
    for g in range(8):
        members = [assign[c][g] for c in range(8)]
        max_out = max(int(ol[b]) for b in members)
        nt = (min(max_out, T) + P - 1) // P
        seg = _fit_segment(members, il, ol, 0, nt, g)
        segs.append(seg)
    for k, seg in enumerate(segs):
        seg.sq_act = k % 2 == 0
    _coverage_check(segs, il, ol)
    return assign, segs


def _build_nc(segs):
    ntt = sum(s.nt for s in segs)
    nf = sum(ln for s in segs for _, ln, _ in s.flags)
    nseg = len(segs)
    nc = bacc.Bacc(None, target_bir_lowering=False)
    attn = nc.declare_dram_parameter("attn", [FLAT], F32, isOutput=False)
    center_d = nc.declare_dram_parameter("center", [P, ntt], F32, isOutput=False)
    thr_d = nc.declare_dram_parameter("thr", [P, max(nf, 1)], F32, isOutput=False)
    acc_d = nc.declare_dram_parameter("acc", [P, nseg], F32, isOutput=True)

    with tile.TileContext(nc) as tc:
        with (
            tc.tile_pool(name="const", bufs=1) as const_pool,
            tc.tile_pool(name="at", bufs=3) as at_pool,
            tc.tile_pool(name="d", bufs=3) as d_pool,
            tc.tile_pool(name="g", bufs=3) as g_pool,
            tc.tile_pool(name="cm", bufs=3) as cm_pool,
            tc.tile_pool(name="junk", bufs=3) as junk_pool,
        ):
            w_i32 = const_pool.tile([P, E], mybir.dt.int32, tag="w_i32")
            w_f32 = const_pool.tile([P, E], F32, tag="w_f32")
            center = const_pool.tile([P, ntt], F32, tag="center")
            thr = const_pool.tile([P, max(nf, 1)], F32, tag="thr")
            acc = const_pool.tile([P, nseg], F32, tag="acc")

            nc.gpsimd.iota(w_i32[:], pattern=[[1, E]], base=0, channel_multiplier=0)
            nc.vector.tensor_copy(w_f32[:], w_i32[:])
            nc.gpsimd.memset(acc[:], 0.0)
            nc.sync.dma_start(out=center[:], in_=center_d[:])
            if nf:
                nc.sync.dma_start(out=thr[:], in_=thr_d[:])

            k0 = 0
            f0 = 0
            for si, seg in enumerate(segs):
                nt, W = seg.nt, seg.W
                fw = nt * W
                at = at_pool.tile([P, fw], F32, tag="at")
                src = AP(
                    attn[:].tensor,
                    PADF + seg.g * T * E + seg.t0 * P * E + seg.b,
                    [
                        [E + seg.a2, P],
                        [P * E + seg.at, nt],
                        [1, W],
                    ],
                )
                dst = at[:]
                nc.sync.dma_start(
                    out=AP(dst.tensor, dst.offset, [dst.ap[0], [W, nt], [1, W]]),
                    in_=src,
                )
                # d = w - center  (broadcast w over t, center over w)
                d = d_pool.tile([P, fw], F32, tag="d")
                wap = w_f32[:, 0:W]
                w_b = AP(wap.tensor, wap.offset, [wap.ap[0], [0, nt], [1, W]])
                cap = center[:, k0 : k0 + nt]
                c_b = AP(cap.tensor, cap.offset, [cap.ap[0], [1, nt], [0, W]])
                dap = d[:]
                d3 = AP(dap.tensor, dap.offset, [dap.ap[0], [W, nt], [1, W]])
                nc.vector.tensor_tensor(d3, w_b, c_b, OP.subtract)
                # threshold masks on flagged tile-runs: one compare (thr
                # broadcast over w) + one fold per run
                for t0r, ln, kind in seg.flags:
                    cm = cm_pool.tile([P, ln * W], F32, tag="cm")
                    cmap = cm[:]
                    cm3 = AP(cmap.tensor, cmap.offset,
                             [cmap.ap[0], [W, ln], [1, W]])
                    w_br = AP(wap.tensor, wap.offset,
                              [wap.ap[0], [0, ln], [1, W]])
                    tap = thr[:, f0 : f0 + ln]
                    thr_b = AP(tap.tensor, tap.offset,
                               [tap.ap[0], [1, ln], [0, W]])
                    op = OP.is_lt if kind == "A" else OP.is_ge
                    nc.vector.tensor_tensor(cm3, w_br, thr_b, op)
                    dsl = d[:, t0r * W : (t0r + ln) * W]
                    nc.vector.scalar_tensor_tensor(
                        dsl, cm[:], 3e4, dsl, OP.mult, OP.add,
                    )
                    f0 += ln
                # d2, g, reduce
                d2 = junk_pool.tile([P, fw], F32, tag="d2")
                nc.scalar.activation(d2[:], d[:], AF.Square)
                gt = g_pool.tile([P, fw], F32, tag="gt")
                nc.scalar.activation(gt[:], d2[:], AF.Exp, scale=NEG_SCALE)
                jk = junk_pool.tile([P, fw], F32, tag="jk")
                nc.vector.scalar_tensor_tensor(
                    jk[:], gt[:], 1.0, at[:], OP.mult, OP.mult,
                    accum_out=acc[:, si : si + 1],
                )
                k0 += nt
            nc.sync.dma_start(out=acc_d[:], in_=acc[:])
    return nc


def _get_nc(segs):
    key = tuple(s.key() for s in segs)
    if key not in _NC_CACHE:
        nc = _build_nc(segs)
        if not nc.is_finalized():
            nc.finalize()
        _NC_CACHE[key] = nc
    return _NC_CACHE[key]


def _make_tables(il, ol, assign_c, segs):
    ntt = sum(s.nt for s in segs)
    nf = sum(ln for s in segs for _, ln, _ in s.flags)
    center = np.full((P, ntt), 1e4, np.float32)
    thr = np.zeros((P, max(nf, 1)), np.float32)
    k0 = 0
    f0 = 0
    for seg in segs:
        b = assign_c[seg.g]
        o, n = int(ol[b]), int(il[b])
        lim = min(n, E)
        rows = seg.nt * P
        i = seg.t0 * P + np.arange(rows)
        idl = _ideal_f32(i, n, o)
        validr = i < min(o, T)
        cen = np.where(validr, idl - seg.sigma.astype(np.float32), np.float32(1e4))
        center[:, k0 : k0 + seg.nt] = cen.reshape(seg.nt, P).T
        for t0r, ln, kind in seg.flags:
            for t in range(t0r, t0r + ln):
                sg = seg.sigma[t * P : (t + 1) * P]
                if kind == "A":
                    thr[:, f0] = (-sg).astype(np.float32)
                else:
                    thr[:, f0] = (lim - sg).astype(np.float32)
                f0 += 1
        k0 += seg.nt
    return {"center": center, "thr": thr}


def _run(attention_weights, input_lengths, output_lengths, **spmd_kwargs):
    attention_weights = np.ascontiguousarray(attention_weights, dtype=np.float32)
    il = np.asarray(input_lengths, dtype=np.int64)
    ol = np.asarray(output_lengths, dtype=np.int64)
    assign, segs = _build_schedule(il, ol)
    in_maps = []
    for c in range(N_CORES):
        flat = np.empty(FLAT, np.float32)
        flat[:PADF] = 0.0
        flat[PADF : PADF + B_LOC * T * E] = attention_weights[assign[c]].reshape(-1)
        flat[PADF + B_LOC * T * E :] = 0.0
        in_maps.append(
            {"attn": flat, **_make_tables(il, ol, assign[c], segs)}
        )
    res = run_bass_kernel_spmd(
        _get_nc(segs), in_maps, list(range(N_CORES)), **spmd_kwargs
    )
    total = sum(float(r["acc"].sum(dtype=np.float64)) for r in res.results)
    return np.float32(total / float(B * T * E)), res


def kernel(attention_weights, input_lengths, output_lengths):
    out, _ = _run(attention_weights, input_lengths, output_lengths)
    return out


# revision 21
# speedup vs baseline: 4.4333x; 1.5949x over previous
"""GuidedAttentionLoss on Trainium2 — 8 NeuronCores, diagonal-band gather.

loss = mean(attention_weights * mask), mask[b,i,j] =
    (i < out_len_b) & (j < in_len_b) ? exp(-(j - floor(i/out*in))^2 / (2*0.4^2)) : 0

With sigma=0.4 the Gaussian underflows to exactly 0 in f32 beyond
|j - ideal_i| ~ 4.6, so per valid row only a ~9-wide band of columns can
contribute. Strategy:

- Batches are sorted by slope in/out and dealt into 8 slot-columns of 8
  (one batch per core per slot) -> pure SPMD: every core runs the identical
  program; per-core data (attention shard + mask tables) differs.
- Per column a quantized-affine "shear line" sigma(i) = a1*p1 + a2*p2 + at*t
  + b tracks ideal(i); a single 4-dim DMA access pattern
  [[3200+a1,16],[400+a2,8],[51200+at,nt],[1,W]] gathers the whole column's
  band ([128 rows/tile] x [W cols], nt tiles) in ONE DMA instruction.
  W is sized exactly on the host from the union of the 8 members' needs.
- Mask math per column, consolidated over the whole [128, nt*W] tile:
    d = w_iota - center   (DVE; center[p,t] = ideal - sigma, +1e4 if invalid)
    d2 = d*d              (ACT Square or DVE mult, alternating)
    g = exp(-3.125*d2)    (ACT)
    acc[:,s] += g*attn    (DVE stt accum)
  Garbage positions (front spill j<0, j>=min(in,400)) that land within 8 of a
  valid ideal are masked by per-tile threshold compare+fold ops; everything
  else dies in the Gaussian underflow. Host sums acc in f64.
"""

import numpy as np

import concourse.bacc as bacc
import concourse.bass as bass  # noqa: F401
import concourse.mybir as mybir
from concourse.ap import AP
from concourse import tile
from concourse.bass_utils import run_bass_kernel_spmd

N_CORES = 8
B, T, E = 64, 2000, 400
B_LOC = B // N_CORES
P = 128
D = 4       # band half-width kept exactly
PROX = 8    # garbage within this of a valid ideal must be masked
PADF = 512
PADB = 81920
FLAT = PADF + B_LOC * T * E + PADB
NEG_SCALE = -3.125
F32 = mybir.dt.float32
AF = mybir.ActivationFunctionType
OP = mybir.AluOpType

_NC_CACHE = {}


def _ideal_f32(i, in_len, out_len):
    safe_out = np.float32(max(float(out_len), 1.0))
    return np.floor((i.astype(np.float32) / safe_out) * np.float32(in_len)).astype(
        np.float32
    )


class _Seg:
    __slots__ = ("g", "members", "t0", "nt", "W", "mode", "a2", "at", "b",
                 "sigma", "flags", "sq_act")

    def key(self):
        return (self.g, self.t0, self.nt, self.W, self.mode, self.a2,
                self.at, self.b, tuple(self.flags), self.sq_act)


def _fit_segment(members, il, ol, t0, nt, g):
    """Fit shear line + W for rows [t0*128, (t0+nt)*128) of slot g."""
    seg = _Seg()
    seg.g = g
    seg.members = members
    seg.t0 = t0
    seg.nt = nt
    rows = nt * P
    i = t0 * P + np.arange(rows)
    A = np.full((8, rows), 1e9)
    Bb = np.full((8, rows), -1e9)
    valid = np.zeros((8, rows), bool)
    ideals = np.zeros((8, rows))
    for m, b in enumerate(members):
        o, n = int(ol[b]), int(il[b])
        valid[m] = i < min(o, T)
        idl = _ideal_f32(i, n, o).astype(np.float64)
        ideals[m] = idl
        A[m] = np.maximum(0.0, idl - D)
        Bb[m] = np.minimum(n - 1, idl + D)
    anyv = valid.any(0)
    Amin = np.where(valid, A, 1e9).min(0)
    Bmax = np.where(valid, Bb, -1e9).max(0)

    slopes = [il[b] / max(ol[b], 1) for b in members]
    cands = set()
    for s in set(np.quantile(slopes, [0.0, 0.25, 0.5, 0.75, 1.0])):
        for f1 in (np.floor, np.round):
            for f3 in (np.floor, np.round):
                at3 = int(f3(128 * s))
                for dat in (-1, 0, 1):
                    cands.add((int(f1(s)), at3 + dat))
    rr = np.arange(rows)
    t_idx = rr // P
    p = rr % P

    best = None
    for a2, at in cands:
        sig0 = a2 * p + at * t_idx
        b_off = int(np.floor((Amin - sig0)[anyv].min()))
        W = int(np.ceil((Bmax - sig0)[anyv].max() - b_off)) + 1
        if best is None or W < best[0]:
            best = (W, a2, at, b_off)
    WS = int(Bmax[anyv].max()) + 1
    if WS <= best[0]:
        seg.mode = "S"
        seg.a2 = seg.at = 0
        seg.b = 0
        seg.W = WS
    else:
        seg.mode = "L"
        seg.W, seg.a2, seg.at, seg.b = best
    assert seg.W <= E + PROX, (seg.W, seg.mode)
    seg.sigma = seg.a2 * p + seg.at * t_idx + seg.b
    seg.flags = []
    return seg


def _coverage_check(segs, il, ol):
    for seg in segs:
        rows = seg.nt * P
        i = seg.t0 * P + np.arange(rows)
        for m, b in enumerate(seg.members):
            o, n = int(ol[b]), int(il[b])
            v = i < min(o, T)
            if not v.any():
                continue
            idl = _ideal_f32(i, n, o).astype(np.float64)
            A = np.maximum(0.0, idl - D)
            Bb = np.minimum(n - 1, idl + D)
            ok = (~v) | ((seg.sigma <= A) & (Bb < seg.sigma + seg.W))
            assert ok.all(), (seg.g, b, np.where(~ok)[0][:5])
            # flat addressing bounds
            base = seg.g * T * E + i * E + seg.sigma
            assert (PADF + base).min() >= 0
            assert (PADF + base + seg.W).max() <= FLAT


def _build_schedule(input_lengths, output_lengths):
    il = np.asarray(input_lengths, dtype=np.int64)
    ol = np.asarray(output_lengths, dtype=np.int64)
    slopes = il.astype(np.float64) / np.maximum(ol, 1)
    order = np.argsort(slopes, kind="stable")
    assign = [[int(order[8 * g + c]) for g in range(8)] for c in range(8)]
    segs = []
    for g in range(8):
        members = [assign[c][g] for c in range(8)]
        max_out = max(int(ol[b]) for b in members)
        nt = (min(max_out, T) + P - 1) // P
        seg = _fit_segment(members, il, ol, 0, nt, g)
        segs.append(seg)
    for k, seg in enumerate(segs):
        seg.sq_act = k % 2 == 0
    _coverage_check(segs, il, ol)
    return assign, segs


def _build_nc(segs):
    ntt = sum(s.nt for s in segs)
    nf = sum(ln for s in segs for _, ln, _ in s.flags)
    nseg = len(segs)
    nc = bacc.Bacc(None, target_bir_lowering=False)
    attn = nc.declare_dram_parameter("attn", [FLAT], F32, isOutput=False)
    center_d = nc.declare_dram_parameter("center", [P, ntt], F32, isOutput=False)
    thr_d = nc.declare_dram_parameter("thr", [P, max(nf, 1)], F32, isOutput=False)
    acc_d = nc.declare_dram_parameter("acc", [P, nseg], F32, isOutput=True)

    with tile.TileContext(nc) as tc:
        with (
            tc.tile_pool(name="const", bufs=1) as const_pool,
            tc.tile_pool(name="at", bufs=3) as at_pool,
            tc.tile_pool(name="d", bufs=3) as d_pool,
            tc.tile_pool(name="g", bufs=3) as g_pool,
            tc.tile_pool(name="cm", bufs=3) as cm_pool,
            tc.tile_pool(name="junk", bufs=3) as junk_pool,
        ):
            w_i32 = const_pool.tile([P, E], mybir.dt.int32, tag="w_i32")
            w_f32 = const_pool.tile([P, E], F32, tag="w_f32")
            center = const_pool.tile([P, ntt], F32, tag="center")
            thr = const_pool.tile([P, max(nf, 1)], F32, tag="thr")
            acc = const_pool.tile([P, nseg], F32, tag="acc")

            nc.gpsimd.iota(w_i32[:], pattern=[[1, E]], base=0, channel_multiplier=0)
            nc.vector.tensor_copy(w_f32[:], w_i32[:])
            nc.gpsimd.memset(acc[:], 0.0)
            nc.sync.dma_start(out=center[:], in_=center_d[:])
            if nf:
                nc.sync.dma_start(out=thr[:], in_=thr_d[:])

            k0 = 0
            f0 = 0
            for si, seg in enumerate(segs):
                nt, W = seg.nt, seg.W
                fw = nt * W
                at = at_pool.tile([P, fw], F32, tag="at")
                src = AP(
                    attn[:].tensor,
                    PADF + seg.g * T * E + seg.t0 * P * E + seg.b,
                    [
                        [E + seg.a2, P],
                        [P * E + seg.at, nt],
                        [1, W],
                    ],
                )
                dst = at[:]
                nc.sync.dma_start(
                    out=AP(dst.tensor, dst.offset, [dst.ap[0], [W, nt], [1, W]]),
                    in_=src,
                )
                # d = w - center  (broadcast w over t, center over w)
                d = d_pool.tile([P, fw], F32, tag="d")
                wap = w_f32[:, 0:W]
                w_b = AP(wap.tensor, wap.offset, [wap.ap[0], [0, nt], [1, W]])
                cap = center[:, k0 : k0 + nt]
                c_b = AP(cap.tensor, cap.offset, [cap.ap[0], [1, nt], [0, W]])
                dap = d[:]
                d3 = AP(dap.tensor, dap.offset, [dap.ap[0], [W, nt], [1, W]])
                nc.vector.tensor_tensor(d3, w_b, c_b, OP.subtract)
                # garbage positions (front spill j<0, j>=min(in,400)) are NOT
                # masked on-device: the host subtracts their contribution
                # exactly (boundary rows only)
                d2 = junk_pool.tile([P, fw], F32, tag="d2")
                nc.scalar.activation(d2[:], d[:], AF.Square)
                gt = g_pool.tile([P, fw], F32, tag="gt")
                nc.scalar.activation(gt[:], d2[:], AF.Exp, scale=NEG_SCALE)
                jk = junk_pool.tile([P, fw], F32, tag="jk")
                nc.vector.scalar_tensor_tensor(
                    jk[:], gt[:], 1.0, at[:], OP.mult, OP.mult,
                    accum_out=acc[:, si : si + 1],
                )
                k0 += nt
            nc.sync.dma_start(out=acc_d[:], in_=acc[:])
    return nc


def _get_nc(segs):
    key = tuple(s.key() for s in segs)
    if key not in _NC_CACHE:
        nc = _build_nc(segs)
        if not nc.is_finalized():
            nc.finalize()
        _NC_CACHE[key] = nc
    return _NC_CACHE[key]


def _make_tables(il, ol, assign_c, segs):
    ntt = sum(s.nt for s in segs)
    nf = sum(ln for s in segs for _, ln, _ in s.flags)
    center = np.full((P, ntt), 1e4, np.float32)
    thr = np.zeros((P, max(nf, 1)), np.float32)
    k0 = 0
    f0 = 0
    for seg in segs:
        b = assign_c[seg.g]
        o, n = int(ol[b]), int(il[b])
        lim = min(n, E)
        rows = seg.nt * P
        i = seg.t0 * P + np.arange(rows)
        idl = _ideal_f32(i, n, o)
        validr = i < min(o, T)
        cen = np.where(validr, idl - seg.sigma.astype(np.float32), np.float32(1e4))
        center[:, k0 : k0 + seg.nt] = cen.reshape(seg.nt, P).T
        for t0r, ln, kind in seg.flags:
            for t in range(t0r, t0r + ln):
                sg = seg.sigma[t * P : (t + 1) * P]
                if kind == "A":
                    thr[:, f0] = (-sg).astype(np.float32)
                else:
                    thr[:, f0] = (lim - sg).astype(np.float32)
                f0 += 1
        k0 += seg.nt
    return {"center": center, "thr": thr}


def _run(attention_weights, input_lengths, output_lengths, **spmd_kwargs):
    attention_weights = np.ascontiguousarray(attention_weights, dtype=np.float32)
    il = np.asarray(input_lengths, dtype=np.int64)
    ol = np.asarray(output_lengths, dtype=np.int64)
    assign, segs = _build_schedule(il, ol)
    in_maps = []
    for c in range(N_CORES):
        flat = np.empty(FLAT, np.float32)
        flat[:PADF] = 0.0
        flat[PADF : PADF + B_LOC * T * E] = attention_weights[assign[c]].reshape(-1)
        flat[PADF + B_LOC * T * E :] = 0.0
        in_maps.append(
            {"attn": flat, **_make_tables(il, ol, assign[c], segs)}
        )
    res = run_bass_kernel_spmd(
        _get_nc(segs), in_maps, list(range(N_CORES)), **spmd_kwargs
    )
    total = sum(float(r["acc"].sum(dtype=np.float64)) for r in res.results)
    total -= _garbage_correction(in_maps, il, ol, assign, segs)
    return np.float32(total / float(B * T * E)), res


def _garbage_correction(in_maps, il, ol, assign, segs):
    """Sum of the out-of-range contributions the device wrongly included.

    The device computes g = exp(-3.125*(j-ideal)^2) for every loaded window
    position, including j<0 (front spill into the previous row) and
    j >= min(in_len, 400) (invalid columns / right spill). Only positions
    within ~16 of a valid row's ideal survive the underflow; recompute those
    few terms here and subtract.
    """
    M = 24
    corr = 0.0
    for c in range(N_CORES):
        flat = in_maps[c]["attn"]
        for seg in segs:
            b = assign[c][seg.g]
            o, n = int(ol[b]), int(il[b])
            lim = min(n, E)
            rows = seg.nt * P
            i = seg.t0 * P + np.arange(rows)
            validr = i < min(o, T)
            idl = _ideal_f32(i, n, o).astype(np.float64)
            sg = seg.sigma
            fr = validr & (
                ((sg < 0) & (idl <= M)) | ((sg + seg.W > lim) & (idl >= lim - M))
            )
            if not fr.any():
                continue
            ii = i[fr]
            j = sg[fr][:, None] + np.arange(seg.W)[None, :]
            d = j - idl[fr][:, None]
            bad = ((j < 0) | (j >= lim)) & (np.abs(d) <= M)
            if not bad.any():
                continue
            addr = PADF + seg.g * T * E + ii[:, None] * E + j
            vals = flat[addr[bad]].astype(np.float64)
            corr += float(np.sum(np.exp(-3.125 * d[bad] ** 2) * vals))
    return corr


def kernel(attention_weights, input_lengths, output_lengths):
    out, _ = _run(attention_weights, input_lengths, output_lengths)
    return out
